# revision 1
# baseline (speedup 1.0000x reference)
"""Trainium2 Bass kernel for nn_AttnBlock_80942953660753 (sparse windowed attention block).

Self-contained: host-side Morton sort + permutation + rope-table gathers + weight
folding (exact linear algebra), then a Bass/Tile kernel running the full
LN1 -> QKV -> RoPE -> windowed attention -> proj -> +res -> LN2 -> FFN -> +res
pipeline on 8 NeuronCores (2 attention windows of 1024 tokens per core).

Device layout choices:
  - residual stream kept C-major ([C, tok], fp32), matching the problem I/O
  - q/k produced token-major for cheap RoPE, then PE-transposed into padded
    head-pair layout ([48 rows head A @ base 0, head B @ base 64]) so the
    K=48-contraction score matmuls row-tile into concurrent PE strips
  - softmax without max-subtraction (scores are O(1)), exp on ACT from PSUM,
    denominators via a ones-column appended to V, normalize on DVE
  - all matmuls bf16 (fp32 PSUM accumulation); LN stats from bf16 copies via
    ones-matmuls in token-major [128, nblocks] layout; rsqrt via fp bit-trick
    + Newton iterations on DVE (avoids ACT table switches)
"""

import math

import numpy as np
import ml_dtypes

# --- static problem config (hardcoded per contest rules) ---
B = 1
C = 384
H = 8
HD = 48
K = 1024
N = 16384
NCORES = 8
T = N // NCORES            # 2048 tokens per core
W = T // K                 # 2 windows per core
NT = T // 128              # 16 token tiles per core
NJ = K // 128              # 8 j-tiles per window
NPAIR = H // 2             # 4 head pairs
F1 = 4 * C                 # 1536
MAX_POS = 4096
BASE = 100.0
GRID_SIZE = 0.02
LN_EPS = 1e-5
SCALE = 1.0 / math.sqrt(HD)

BF16 = ml_dtypes.bfloat16

_CACHE = {}


# ----------------------------------------------------------------------------
# host-side helpers (exact numpy re-implementation of the serialization)
# ----------------------------------------------------------------------------

def _interleave8(x, y, z):
    key = np.zeros_like(x)
    for i in range(8):
        m = 1 << i
        key = key | ((x & m) << (2 * i + 2)) | ((y & m) << (2 * i + 1)) | ((z & m) << (2 * i))
    return key


def _serialize_points(xyz):
    pts = np.transpose(xyz, (0, 2, 1)).astype(np.float32, copy=False)
    mn = pts.min(axis=1, keepdims=True)
    grid = ((pts - mn) / np.float32(GRID_SIZE)).astype(np.int32)
    grid = np.clip(grid, 0, 65535)
    gx, gy, gz = grid[..., 0], grid[..., 1], grid[..., 2]
    lo = _interleave8(gx & 255, gy & 255, gz & 255)
    hi = _interleave8((gx >> 8) & 255, (gy >> 8) & 255, (gz >> 8) & 255)
    order = np.stack([np.lexsort((lo[b], hi[b])) for b in range(pts.shape[0])])
    return order.astype(np.int32), grid


def _rope_tok_tables(grid_sorted):
    """cos/sign-folded-sin per token, (N, 48) fp32."""
    d = HD // 3
    inv_freq = 1.0 / (np.float32(BASE) ** (np.arange(0, d, 2, dtype=np.float32) / np.float32(d)))
    t = np.arange(MAX_POS, dtype=np.float32)
    freqs = t[:, None] * inv_freq[None, :]
    freqs = np.concatenate([freqs, freqs], axis=1)          # (MAX_POS, 16)
    cos_t = np.cos(freqs)
    sin_t = np.sin(freqs)
    pos = np.clip(grid_sorted, 0, MAX_POS - 1)
    cos_tok = np.concatenate([cos_t[pos[:, a]] for a in range(3)], axis=1)   # (N,48)
    sin_tok = np.concatenate([sin_t[pos[:, a]] for a in range(3)], axis=1)
    ss_tok = sin_tok.copy()
    for g in range(3):
        ss_tok[:, g * 16:g * 16 + 8] *= -1.0
    return cos_tok, ss_tok


def _tok_tile_layout(arr_t48):
    """(T,48) -> [128, NT*48] with col tt*48+d holding token tt*128+p."""
    return np.ascontiguousarray(
        arr_t48.reshape(NT, 128, 48).transpose(1, 0, 2).reshape(128, NT * 48))


# ----------------------------------------------------------------------------
# device kernel emission
# ----------------------------------------------------------------------------

def _build(debug_dump=False):
    import contextlib

    import concourse.bass as bass
    import concourse.tile as tile
    from concourse import bacc, mybir

    f32 = mybir.dt.float32
    bf16 = mybir.dt.bfloat16

    nc = bacc.Bacc("TRN2", target_bir_lowering=False, debug=False, num_devices=NCORES)

    # ---- DRAM I/O ----
    dram = {}
    dram["xT"] = nc.dram_tensor("xT", [C, T], f32, kind="ExternalInput").ap()
    dram["cosT"] = nc.dram_tensor("cosT", [128, NT * 48], f32, kind="ExternalInput").ap()
    dram["ssT"] = nc.dram_tensor("ssT", [128, NT * 48], f32, kind="ExternalInput").ap()
    dram["wqk"] = nc.dram_tensor("wqk", [C, 2 * C], bf16, kind="ExternalInput").ap()
    dram["wv"] = nc.dram_tensor("wv", [C, C], bf16, kind="ExternalInput").ap()
    dram["wproj"] = nc.dram_tensor("wproj", [C, C], bf16, kind="ExternalInput").ap()
    dram["w1"] = nc.dram_tensor("w1", [C, F1], bf16, kind="ExternalInput").ap()
    dram["w2"] = nc.dram_tensor("w2", [F1, C], bf16, kind="ExternalInput").ap()
    dram["ident"] = nc.dram_tensor("ident", [128, 128], bf16, kind="ExternalInput").ap()
    dram["outT"] = nc.dram_tensor("outT", [C, T], f32, kind="ExternalOutput").ap()
    dram["dumps"] = {}
    if debug_dump:
        for nm, shape, dt in [("xn", [C, T], bf16), ("qr", [128, NPAIR * K], bf16),
                              ("kr", [128, NPAIR * K], bf16), ("attno", [128, NT * C], bf16),
                              ("rs1", [128, NT], f32), ("qkt", [128, K], bf16)]:
            dram["dumps"][nm] = nc.dram_tensor("dump_" + nm, shape, dt, kind="ExternalOutput").ap()

    with tile.TileContext(nc) as tc:
        with contextlib.ExitStack() as ctx:
            _emit(tc, nc, ctx, bass, tile, mybir, dram)
    nc.compile()
    return nc


def _emit(tc, nc, ctx, bass, tile, mybir, dram):
    f32 = mybir.dt.float32
    bf16 = mybir.dt.bfloat16
    i32 = mybir.dt.int32
    ALU = mybir.AluOpType
    ACTF = mybir.ActivationFunctionType
    AP = bass.AP

    d_out = dram["outT"]
    dumps = dram["dumps"]

    # ---------------- pools ----------------
    const = ctx.enter_context(tc.tile_pool(name="const", bufs=1))
    persist = ctx.enter_context(tc.tile_pool(name="persist", bufs=1))
    workLN = ctx.enter_context(tc.tile_pool(name="workLN", bufs=2))
    qkt_pool = ctx.enter_context(tc.tile_pool(name="qkt", bufs=9))
    p_pool = ctx.enter_context(tc.tile_pool(name="ptile", bufs=16))
    small = ctx.enter_context(tc.tile_pool(name="small", bufs=8))
    outp = ctx.enter_context(tc.tile_pool(name="outp", bufs=2))
    # QKV-phase pool: released after T2(w1); its space is reused by `late`
    workQKV = tc.alloc_tile_pool(name="workQKV", bufs=1)

    ps_mm = ctx.enter_context(tc.tile_pool(name="psmm", bufs=2, space="PSUM"))
    ps_big = ctx.enter_context(tc.tile_pool(name="psbig", bufs=3, space="PSUM"))
    dram_pool = ctx.enter_context(tc.tile_pool(name="drs", bufs=4, space="DRAM"))

    # ---------------- constants / weights ----------------
    ident = const.tile([128, 128], bf16)
    nc.sync.dma_start(ident[:], dram["ident"][:])
    ident32 = const.tile([128, 128], f32)
    nc.scalar.copy(ident32[:], ident[:])
    onesb = const.tile([128, 1], bf16)
    nc.vector.memset(onesb[:], 1.0)

    xT = [persist.tile([128, T], f32, tag=f"xT{cc}", name=f"xT{cc}") for cc in range(3)]
    for half in range(2):
        for cc in range(3):
            nc.sync.dma_start(xT[cc][:, half * K:(half + 1) * K],
                              dram["xT"][cc * 128:(cc + 1) * 128, half * K:(half + 1) * K])

    wqk = [workQKV.tile([128, 2 * C], bf16, tag="wqk", bufs=3, name=f"wqk{i}") for i in range(3)]
    wv = [workQKV.tile([128, C], bf16, tag="wv", bufs=3, name=f"wv{i}") for i in range(3)]
    wp = [const.tile([128, C], bf16, tag="wp", bufs=3, name=f"wp{i}") for i in range(3)]
    w1 = [const.tile([128, F1], bf16, tag="w1", bufs=3, name=f"w1_{i}") for i in range(3)]
    for cc in range(3):
        nc.sync.dma_start(wqk[cc][:], dram["wqk"][cc * 128:(cc + 1) * 128, :])
        nc.sync.dma_start(wv[cc][:], dram["wv"][cc * 128:(cc + 1) * 128, :])
        nc.sync.dma_start(wp[cc][:], dram["wproj"][cc * 128:(cc + 1) * 128, :])
        nc.sync.dma_start(w1[cc][:], dram["w1"][cc * 128:(cc + 1) * 128, :])

    cosT = workQKV.tile([128, NT * 48], f32, tag="cos")
    ssT = workQKV.tile([128, NT * 48], f32, tag="ss")
    nc.sync.dma_start(cosT[:], dram["cosT"][:])
    nc.sync.dma_start(ssT[:], dram["ssT"][:])


    # xnT doubles as attnoT once QKV is done with it
    xnT = [persist.tile([128, T], bf16, tag=f"xnT{cc}", name=f"xnT{cc}") for cc in range(3)]
    attnoT = xnT
    xn2T = [persist.tile([128, T], bf16, tag=f"xn2T{cc}", name=f"xn2T{cc}") for cc in range(3)]
    attno = persist.tile([128, NT * C], bf16, tag="attno")
    vpk = persist.tile([128, NT * (HD + 1) * H], bf16, tag="vpk")
    vpk_ones = AP(vpk[:].tensor, vpk[:].offset + 48,
                  [vpk[:].ap[0], [392, NT], [49, H], [1, 1]])
    nc.gpsimd.memset(vpk_ones, 1.0)

    def copy_on(eng, out, in_):
        if eng is nc.scalar:
            eng.copy(out, in_)
        else:
            eng.tensor_copy(out, in_)

    # ---------------- LayerNorm helper ----------------
    def layernorm(src_tiles, dst_tiles, tok0, ntok, dump_rs=None, apply_eng=None):
        """src fp32 [128, T] C-major tiles -> dst bf16 tiles, tokens [tok0, tok0+ntok)."""
        nb = ntok // 128
        nch = ntok // 512
        stats = ps_mm.tile([128, 2 * nb], f32, tag="mm", name="stats")
        sums = stats[:, 0:nb]
        sqs = stats[:, nb:2 * nb]
        for tch in range(nch):
            xb = [workLN.tile([128, 512], bf16, tag="lnxb", bufs=3, name=f"lnxb{i}")
                  for i in range(3)]
            x2b = [workLN.tile([128, 512], bf16, tag="lnx2b", bufs=3, name=f"lnx2b{i}")
                   for i in range(3)]
            for cc in range(3):
                sl = slice(tok0 + tch * 512, tok0 + (tch + 1) * 512)
                nc.vector.tensor_copy(xb[cc][:], src_tiles[cc][:, sl])
                nc.vector.tensor_tensor(x2b[cc][:], xb[cc][:], xb[cc][:], ALU.mult)
            for bl in range(4):
                b = tch * 4 + bl
                for cc in range(3):
                    nc.tensor.matmul(sums[:, b:b + 1], xb[cc][:, bl * 128:(bl + 1) * 128],
                                     onesb[:], start=(cc == 0), stop=(cc == 2),
                                     skip_group_check=True)
                for cc in range(3):
                    nc.tensor.matmul(sqs[:, b:b + 1], x2b[cc][:, bl * 128:(bl + 1) * 128],
                                     onesb[:], start=(cc == 0), stop=(cc == 2),
                                     skip_group_check=True)
        s_sb = small.tile([128, nb], f32, tag="s_sb", name="s_sb")
        q_sb = small.tile([128, nb], f32, tag="q_sb", name="q_sb")
        nc.vector.tensor_copy(s_sb[:], sums)
        nc.vector.tensor_copy(q_sb[:], sqs)
        # var = q/C - (s/C)^2 ; rs = rsqrt(var + eps) via fp bit trick + Newton
        v_sb = small.tile([128, nb], f32, tag="v_sb", name="v_sb")
        t_sb = small.tile([128, nb], f32, tag="t_sb", name="t_sb")
        rsm = small.tile([128, 2 * nb], f32, tag="rsm", name="rsm")
        rs_sb = rsm[:, 0:nb]
        mrs_sb = rsm[:, nb:2 * nb]
        nc.vector.scalar_tensor_tensor(t_sb[:], s_sb[:], 1.0 / (C * C), s_sb[:],
                                       ALU.mult, ALU.mult)             # s^2/C^2
        nc.vector.scalar_tensor_tensor(v_sb[:], q_sb[:], 1.0 / C, t_sb[:],
                                       ALU.mult, ALU.subtract)         # q/C - s^2/C^2
        nc.vector.tensor_scalar_add(v_sb[:], v_sb[:], LN_EPS)
        f_sb = small.tile([128, nb], f32, tag="f_sb", name="f_sb")
        nc.vector.tensor_copy(f_sb[:], v_sb[:].bitcast(i32))           # float(bits(v))
        MAGICF = 1.5 * (127.0 - 0.0450466) * 8388608.0
        nc.vector.tensor_scalar(f_sb[:], f_sb[:], -0.5, MAGICF, op0=ALU.mult, op1=ALU.add)
        yi_sb = small.tile([128, nb], i32, tag="yi_sb", name="yi_sb")
        nc.vector.tensor_copy(yi_sb[:], f_sb[:])                       # round to int
        y_ap = yi_sb[:].bitcast(f32)
        for it in range(3):
            nc.vector.tensor_tensor(t_sb[:], y_ap, y_ap, ALU.mult)
            nc.vector.tensor_tensor(t_sb[:], t_sb[:], v_sb[:], ALU.mult)
            nc.vector.tensor_scalar(t_sb[:], t_sb[:], -0.5, 1.5, op0=ALU.mult, op1=ALU.add)
            if it < 2:
                nc.vector.tensor_tensor(y_ap, y_ap, t_sb[:], ALU.mult)
            else:
                nc.vector.tensor_tensor(rs_sb, y_ap, t_sb[:], ALU.mult)
        nc.vector.scalar_tensor_tensor(mrs_sb, s_sb[:], 1.0 / C, rs_sb,
                                       ALU.mult, ALU.mult)             # mu*rs
        if dump_rs is not None:
            nc.sync.dma_start(dump_rs, rs_sb)
        # per 512-token chunk: reshape [128,4] -> row [1,512] via a DRAM bounce
        # (token t = b*128 + p), broadcast across partitions, then apply
        for tch in range(nch):
            rs_row = small.tile([1, 512], f32, tag="rs_row", bufs=2, name="rs_row")
            mrs_row = small.tile([1, 512], f32, tag="mrs_row", bufs=2, name="mrs_row")
            for off, row in ((0, rs_row), (nb, mrs_row)):
                scr = dram_pool.tile([1, 512], f32, tag="scr", bufs=4, name="scr")
                scat = AP(scr[:].tensor, scr[:].offset, [[512, 1], [1, 128], [128, 4]])
                nc.gpsimd.dma_start(scat, rsm[:, off + tch * 4: off + (tch + 1) * 4])
                nc.gpsimd.dma_start(row[:], scr[:])
            rs_bc = workLN.tile([128, 512], f32, tag="rs_bc", bufs=2, name="rs_bc")
            mrs_bc = workLN.tile([128, 512], f32, tag="mrs_bc", bufs=2, name="mrs_bc")
            nc.gpsimd.partition_broadcast(rs_bc[:], rs_row[:])
            nc.gpsimd.partition_broadcast(mrs_bc[:], mrs_row[:])
            sl = slice(tok0 + tch * 512, tok0 + (tch + 1) * 512)
            eng = apply_eng or nc.vector
            for cc in range(3):
                eng.tensor_tensor(dst_tiles[cc][:, sl], src_tiles[cc][:, sl],
                                  rs_bc[:], ALU.mult)
                eng.tensor_tensor(dst_tiles[cc][:, sl], dst_tiles[cc][:, sl],
                                  mrs_bc[:], ALU.subtract)

    # ---------------- LN1 (first window; second window interleaved later) ----------------
    layernorm(xT, xnT, 0, K,
              dump_rs=dumps["rs1"][:, 0:NJ] if dumps else None)
    if dumps:
        for cc in range(3):
            nc.sync.dma_start(dumps["xn"][cc * 128:(cc + 1) * 128, :], xnT[cc][:])

    # ---------------- QKV + rope (token-major) + T2 via DRAM-bounced transposes ----
    qpad = {}
    kpad = {}
    qkT = {}  # (w, pair, 'q'/'k') -> [d, tok] tile [128, K]

    stg_d = {}

    def emit_qkv_prep(w):
        qpad[w] = workQKV.tile([128, NPAIR * K], bf16, tag="qpad", name=f"qpad{w}")
        kpad[w] = workQKV.tile([128, NPAIR * K], bf16, tag="kpad", name=f"kpad{w}")
        stg_d[(w, "q")] = dram_pool.tile([NPAIR * K, 128], bf16, tag="qstg", bufs=2,
                                       name=f"qstg{w}")
        stg_d[(w, "k")] = dram_pool.tile([NPAIR * K, 128], bf16, tag="kstg", bufs=2,
                                       name=f"kstg{w}")
        for p in range(NPAIR):
            for nm in ("k", "q"):
                qkT[(w, p, nm)] = qkt_pool.tile(
                    [128, K], bf16, tag="qkt", name=f"qkt_{w}_{p}_{nm}")

    def emit_qkv_jts(w, jts, vdrain_eng=None):
        qstg, kstg = stg_d[(w, "q")], stg_d[(w, "k")]
        wqk_q = [wqk[cc][:, 0:C] for cc in range(3)]
        wqk_k = [wqk[cc][:, C:2 * C] for cc in range(3)]
        wv_r = [wv[cc][:] for cc in range(3)]
        for jt in jts:
            tt = w * NJ + jt
            cos_sl = AP(cosT[:].tensor, cosT[:].offset + tt * 48,
                        [cosT[:].ap[0], [0, H], [1, HD]])
            for which, pad, stg in (("q", qpad[w], qstg), ("k", kpad[w], kstg),
                                    ("v", None, None)):
                ps = ps_mm.tile([128, C], f32, tag="mm", name=f"{which}ps")
                rhs = {"q": wqk_q, "k": wqk_k, "v": wv_r}[which]
                for cc in range(3):
                    nc.tensor.matmul(ps[:], xnT[cc][:, tt * 128:(tt + 1) * 128],
                                     rhs[cc], start=(cc == 0), stop=(cc == 2))
                if which == "v":
                    v_out = AP(vpk[:].tensor, vpk[:].offset + tt * 392,
                               [vpk[:].ap[0], [49, H], [1, HD]])
                    v_in = AP(ps[:].tensor, ps[:].offset, [ps[:].ap[0], [HD, H], [1, HD]])
                    copy_on(vdrain_eng or nc.scalar, v_out, v_in)
                    continue
                hd_in = AP(ps[:].tensor, ps[:].offset, [ps[:].ap[0], [HD, H], [1, HD]])
                qc = workLN.tile([128, C], bf16, tag="ropec", bufs=2, name="ropec")
                qc_ap = AP(qc[:].tensor, qc[:].offset, [qc[:].ap[0], [HD, H], [1, HD]])
                nc.vector.tensor_tensor(qc_ap, hd_in, cos_sl, ALU.mult)
                qp = workLN.tile([128, C], bf16, tag="ropep", bufs=2, name="ropep")
                for half in range(2):
                    # destination cols: 48h + 16g + (half? 8..15 : 0..7)
                    dst = AP(qp[:].tensor, qp[:].offset + half * 8,
                             [qp[:].ap[0], [HD, H], [16, 3], [1, 8]])
                    sc = AP(ps[:].tensor, ps[:].offset + (8 if half == 0 else 0),
                            [ps[:].ap[0], [HD, H], [16, 3], [1, 8]])
                    ssl = AP(ssT[:].tensor, ssT[:].offset + tt * 48 + half * 8,
                             [ssT[:].ap[0], [0, H], [16, 3], [1, 8]])
                    nc.vector.tensor_tensor(dst, sc, ssl, ALU.mult)
                # pass3: pad[pair layout] = qc + qp   (gpsimd, SBUF-only)
                dst = AP(pad[:].tensor, pad[:].offset + jt * 128,
                         [pad[:].ap[0], [K, NPAIR], [64, 2], [1, HD]])
                in0 = AP(qc[:].tensor, qc[:].offset,
                         [qc[:].ap[0], [96, NPAIR], [HD, 2], [1, HD]])
                in1 = AP(qp[:].tensor, qp[:].offset,
                         [qp[:].ap[0], [96, NPAIR], [HD, 2], [1, HD]])
                nc.gpsimd.tensor_tensor(dst, in0, in1, ALU.add)
                # stage this jt's padded block to DRAM: row p*K + jt*128 + tok
                stg_out = AP(stg[:].tensor, stg[:].offset + (jt * 128) * 128,
                             [[128, 128], [K * 128, NPAIR], [1, 128]])
                stg_in = AP(pad[:].tensor, pad[:].offset + jt * 128,
                            [pad[:].ap[0], [K, NPAIR], [1, 128]])
                nc.sync.dma_start(stg_out, stg_in)
    def emit_qkv_transposes(w):
        # big transposes: [K, 128] dram -> [128, K] sbuf per (pair, q/k)
        for p in range(NPAIR):
            for nm in ("k", "q"):
                s = stg_d[(w, nm)]
                src_ap = AP(s[:].tensor, s[:].offset + p * K * 128,
                            [[128, K], [1, 128]])
                nc.sync.dma_start_transpose(qkT[(w, p, nm)][:], src_ap)

    def emit_qkv_window(w, vdrain_eng=None):
        emit_qkv_prep(w)
        emit_qkv_jts(w, range(NJ), vdrain_eng)
        emit_qkv_transposes(w)

    # ---------------- attention (software-pipelined units) ----------------
    attn_state = {"pending_pv": None}

    def emit_scores_exp(w, p, hh):
        kT = qkT[(w, p, "k")]
        qT = qkT[(w, p, "q")]
        base = 64 * hh
        pts = []
        for jt in range(NJ):
            spair = ps_big.tile([128, K], f32, tag="spair", name="spair")
            for ih in range(2):
                nc.tensor.matmul(spair[:, ih * 512:(ih + 1) * 512],
                                 kT[base:base + HD, jt * 128:(jt + 1) * 128],
                                 qT[base:base + HD, ih * 512:(ih + 1) * 512],
                                 start=True, stop=True, skip_group_check=True)
            pt = p_pool.tile([128, K], bf16, tag="ptile", name=f"pt_{w}_{p}_{jt}_{hh}")
            nc.scalar.activation(pt[:], spair[:], ACTF.Exp, scale=SCALE)
            pts.append(pt)
        return pts

    def emit_pv(w, p, hh, pts):
        h = 2 * p + hh
        for it in range(NJ):
            pv = ps_mm.tile([128, 64], f32, tag="mm", name="pv")
            for jt in range(NJ):
                tt = w * NJ + jt
                nc.tensor.matmul(pv[:, 0:HD + 1],
                                 pts[jt][:, it * 128:(it + 1) * 128],
                                 vpk[:, tt * 392 + 49 * h: tt * 392 + 49 * h + HD + 1],
                                 start=(jt == 0), stop=(jt == NJ - 1))
            rec = small.tile([128, 1], f32, tag="rec", name="rec")
            nc.vector.reciprocal(rec[:], pv[:, HD:HD + 1])
            ti = w * NJ + it
            nc.vector.tensor_scalar_mul(
                attno[:, ti * C + h * HD: ti * C + h * HD + HD],
                pv[:, 0:HD], rec[:])

    def flush_pv():
        if attn_state["pending_pv"] is not None:
            emit_pv(*attn_state["pending_pv"])
            attn_state["pending_pv"] = None

    def emit_attn_halfunit(w, p, hh):
        """Emit pending PV first (pipeline), then this half-unit's scores+exp."""
        flush_pv()
        pts = emit_scores_exp(w, p, hh)
        attn_state["pending_pv"] = (w, p, hh, pts)

    # ---------------- T3 + proj + residual ----------------
    def emit_t3(w, drain_eng=None):
        for tt in range(w * NJ, (w + 1) * NJ):
            for cc in range(3):
                tps = ps_mm.tile([128, 128], bf16, tag="mm", name="tps3")
                nc.tensor.transpose(tps[:], attno[:, tt * C + cc * 128: tt * C + (cc + 1) * 128],
                                    ident[:])
                copy_on(drain_eng or nc.vector,
                        attnoT[cc][:, tt * 128:(tt + 1) * 128], tps[:])

    def emit_proj(w):
        for cc in range(3):
            for tcc in range(2 * w, 2 * w + 2):
                pps = ps_mm.tile([128, 512], f32, tag="mm", name="pps")
                for ci in range(3):
                    nc.tensor.matmul(pps[:], wp[ci][:, cc * 128:(cc + 1) * 128],
                                     attnoT[ci][:, tcc * 512:(tcc + 1) * 512],
                                     start=(ci == 0), stop=(ci == 2))
                nc.vector.tensor_tensor(xT[cc][:, tcc * 512:(tcc + 1) * 512],
                                        xT[cc][:, tcc * 512:(tcc + 1) * 512],
                                        pps[:], ALU.add)

    # ---------------- FFN ----------------
    late = {}

    def open_late_pool():
        late["pool"] = tc.alloc_tile_pool(name="late", bufs=1)
        late["gt"] = late["pool"].tile([128, 12 * K], bf16, tag="gt", name="gt")
        w2t = [late["pool"].tile([128, C], bf16, tag="w2", bufs=12, name=f"w2_{i}")
               for i in range(12)]
        for fc in range(12):
            nc.sync.dma_start(w2t[fc][:], dram["w2"][fc * 128:(fc + 1) * 128, :])
        late["w2"] = w2t

    def emit_ffn_h(w, th_list=(0, 1), gelu_split=False):
        gt = late["gt"]
        for f in range(12):
            hps = ps_big.tile([128, K], f32, tag="spair", name="hps")
            for th in th_list:
                for cc in range(3):
                    nc.tensor.matmul(hps[:, th * 512:(th + 1) * 512],
                                     w1[cc][:, f * 128:(f + 1) * 128],
                                     xn2T[cc][:, w * K + th * 512: w * K + (th + 1) * 512],
                                     start=(cc == 0), stop=(cc == 2), skip_group_check=True)
            if gelu_split:
                for th in th_list:
                    nc.scalar.activation(gt[:, f * K + th * 512: f * K + (th + 1) * 512],
                                         hps[:, th * 512:(th + 1) * 512], ACTF.Gelu)
            else:
                nc.scalar.activation(gt[:, f * K:(f + 1) * K], hps[:], ACTF.Gelu)

    def emit_ffn_y(w, th_list=(0, 1)):
        gt = late["gt"]
        w2t = late["w2"]
        for cc in range(3):
            for th in th_list:
                tcc = 2 * w + th
                yps = ps_mm.tile([128, 512], f32, tag="mm", name="yps")
                for f in range(12):
                    nc.tensor.matmul(yps[:], w2t[f][:, cc * 128:(cc + 1) * 128],
                                     gt[:, f * K + th * 512: f * K + (th + 1) * 512],
                                     start=(f == 0), stop=(f == 11))
                ot = outp.tile([128, 512], f32, tag="out", name=f"ot_{w}_{cc}_{th}")
                nc.vector.tensor_tensor(ot[:], xT[cc][:, tcc * 512:(tcc + 1) * 512],
                                        yps[:], ALU.add)
                nc.sync.dma_start(d_out[cc * 128:(cc + 1) * 128, tcc * 512:(tcc + 1) * 512],
                                  ot[:])

    # ---------------- schedule ----------------
    emit_qkv_window(0, vdrain_eng=nc.scalar)

    # attention w0, pipelined; interleave LN1(w1) + QKV(w1) between half-units
    post_w0_work = [
        lambda: None,
        lambda: layernorm(xT, xnT, K, 512),
        lambda: (layernorm(xT, xnT, K + 512, 512), emit_qkv_prep(1)),
        lambda: emit_qkv_jts(1, [0, 1], vdrain_eng=nc.vector),
        lambda: emit_qkv_jts(1, [2, 3], vdrain_eng=nc.vector),
        lambda: emit_qkv_jts(1, [4, 5], vdrain_eng=nc.vector),
        lambda: (emit_qkv_jts(1, [6, 7], vdrain_eng=nc.vector), emit_qkv_transposes(1)),
        lambda: None,
    ]
    i = 0
    for p in range(NPAIR):
        for hh in range(2):
            emit_attn_halfunit(0, p, hh)
            post_w0_work[i]()
            i += 1

    if dumps:
        nc.sync.dma_start(dumps["qr"][:], qpad[0][:])
        nc.sync.dma_start(dumps["kr"][:], kpad[0][:])
        nc.sync.dma_start(dumps["qkt"][:], qkT[(0, 0, "q")][:])

    workQKV.release()
    open_late_pool()

    # attention w1, pipelined; interleave w0 downstream between half-units
    post_w1_work = [
        lambda: emit_t3(0),
        lambda: emit_proj(0),
        lambda: layernorm(xT, xn2T, 0, 512, apply_eng=nc.gpsimd),
        lambda: layernorm(xT, xn2T, 512, 512, apply_eng=nc.gpsimd),
        lambda: None, lambda: None, lambda: None, lambda: None,
    ]
    i = 0
    for p in range(NPAIR):
        for hh in range(2):
            emit_attn_halfunit(1, p, hh)
            post_w1_work[i]()
            i += 1
    flush_pv()

    if dumps:
        nc.sync.dma_start(dumps["attno"][:], attno[:])

    # tail: h(0) drains behind the last exps; y(0) must run before gelu(1)
    # (shared gt buffer WAR); LN2(w1) chain overlaps y(0) on DVE/GP
    emit_ffn_h(0)
    emit_t3(1)
    emit_proj(1)
    layernorm(xT, xn2T, K, 512)
    layernorm(xT, xn2T, K + 512, 512)
    emit_ffn_y(0)
    emit_ffn_h(1, gelu_split=True)
    emit_ffn_y(1)
    late["pool"].release()


# ----------------------------------------------------------------------------
# host wrapper
# ----------------------------------------------------------------------------

def _prep_inputs(inputs):
    x = np.asarray(inputs["x"], np.float32)
    xyz = np.asarray(inputs["xyz"], np.float32)
    order, grid = _serialize_points(xyz)
    o = order[0]
    xs = x[0][:, o]                                  # (C, N) sorted
    grid_s = grid[0][o]
    cos_tok, ss_tok = _rope_tok_tables(grid_s)

    g1 = np.asarray(inputs["ln1_g"], np.float32)
    b1 = np.asarray(inputs["ln1_b"], np.float32)
    g2 = np.asarray(inputs["ln2_g"], np.float32)
    b2 = np.asarray(inputs["ln2_b"], np.float32)
    w_qkv = np.asarray(inputs["qkv_w"], np.float32)
    for nm in ("qkv_b", "proj_b", "ffn_b1", "ffn_b2"):
        assert np.all(np.asarray(inputs[nm]) == 0.0), f"nonzero bias {nm} unsupported"
    assert np.all(b1 == 0.0) and np.all(b2 == 0.0), "nonzero LN beta unsupported"

    wqk = (g1[:, None] * w_qkv[:, :2 * C]).astype(BF16)
    wv = (g1[:, None] * w_qkv[:, 2 * C:]).astype(BF16)
    wproj = np.asarray(inputs["proj_w"], np.float32).astype(BF16)
    w1 = (g2[:, None] * np.asarray(inputs["ffn_w1"], np.float32)).astype(BF16)
    w2 = np.asarray(inputs["ffn_w2"], np.float32).astype(BF16)
    ident = np.eye(128, dtype=BF16)

    in_maps = []
    for core in range(NCORES):
        sl = slice(core * T, (core + 1) * T)
        in_maps.append({
            "xT": np.ascontiguousarray(xs[:, sl]),
            "cosT": _tok_tile_layout(cos_tok[sl]),
            "ssT": _tok_tile_layout(ss_tok[sl]),
            "wqk": wqk, "wv": wv, "wproj": wproj, "w1": w1, "w2": w2,
            "ident": ident,
        })
    return in_maps, o


def kernel(**inputs):
    from concourse import bass_utils

    key = "nc"
    if key not in _CACHE:
        _CACHE[key] = _build(debug_dump=False)
    nc = _CACHE[key]

    in_maps, o = _prep_inputs(inputs)
    res = bass_utils.run_bass_kernel_spmd(nc, in_maps, core_ids=list(range(NCORES)))
    y = np.concatenate([r["outT"] for r in res.results], axis=1)   # (C, N) sorted order
    out = np.empty((1, C, N), np.float32)
    out[0][:, o] = y
    return out


if __name__ == "__main__":
    # smoke build
    nc = _build()
    print("build OK")

